# revision 19
# baseline (speedup 1.0000x reference)
"""Distributed Adam optimizer step on 8 TRN2 NeuronCores.

Computes the Adam parameter patch for three tensors (conv/mlp/head),
returning the flat concatenation exactly like the reference.

Strategy (pure data-parallel, ZeRO-style): all tensors are flattened and
concatenated into one flat stream of 23,232,512 f32 elements, split evenly
across the 8 cores (2,904,064 each). Each core runs an identical elementwise
Bass/Tile kernel over its chunk; no collectives needed. Scalar hyperparams
are folded on the host into activation scale/bias immediates.

If the moment tensors are degenerate (m == 0 everywhere, v constant — the
case at t=1), an exact algebraic specialization skips loading m and v,
cutting HBM traffic from 5 streams to 3.
"""

import math

import ml_dtypes
import numpy as np

import concourse.bacc as bacc
import concourse.mybir as mybir
from concourse.tile import TileContext
from concourse.bass_utils import run_bass_kernel_spmd

N_CORES = 8
TOTAL = 512 * 512 * 3 * 3 + 4096 * 4096 + 1000 * 4096  # 23,232,512
PER_CORE = TOTAL // N_CORES  # 2,904,064
P = 128
TILE_F = 2836
N_TILES = PER_CORE // (P * TILE_F)  # 8
assert N_TILES * P * TILE_F == PER_CORE

# fp8 e4m3 scale for the gradient stream: g ~ N(0, 0.01), |g| < ~0.08;
# g*G8_SCALE spans [~2e-3, ~170] - inside e4m3's [2^-9 subnormal, 240] range.
G8_SCALE = 2048.0

_ORDER = ("conv", "mlp", "head")

TRACE = False
LAST_RESULT = None

_nc_cache = {}

# The act-table placement pass assigns each ACTIVATE the first table set
# containing its function; Square would first-fit to "exp_and_others" while
# Abs_reciprocal_sqrt lives in "abs_reciprocal_sqrt_and_small", which would
# reload tables twice per tile (~2.6us each). Both functions coexist in
# abs_reciprocal_sqrt_and_small; hide them from every other set (order and
# set count preserved, so act_func_set_ids stay valid) and the whole kernel
# needs exactly one table load.
_orig_get_activation_tables = bacc.get_activation_tables


def _patched_get_activation_tables(arch):
    tables = dict(_orig_get_activation_tables(arch))
    AF = mybir.ActivationFunctionType
    pinned = {AF.Square, AF.Abs_reciprocal_sqrt}
    out = {}
    for name, funcs in tables.items():
        if name == "abs_reciprocal_sqrt_and_small":
            out[name] = funcs
        else:
            out[name] = funcs - pinned
    return out


bacc.get_activation_tables = _patched_get_activation_tables


def _build_fast(ars_scale, b_ars):
    """out = p - g8 * (1/sqrt(ars_scale*g8^2 + b_ars)), p/out bf16, g8 fp8.

    g8 = G8_SCALE*g quantized to e4m3; ars_scale/b_ars absorb G8_SCALE and
    the Adam scalars so the mul's product is the true update term. The
    update saturates at +-A for |g| >> sqrt(C/B), so fp8's coarse mantissa
    on g contributes negligible output error; p/out bf16 dominates at
    ~1e-3 norm relative error, well inside the 2e-2 gate. The rsqrt is the
    Abs_reciprocal_sqrt ACT table function (1 elem/cycle) instead of DVE
    reciprocal (~6 cycles/elem)."""
    nc = bacc.Bacc(None, target_bir_lowering=False)
    f32 = mybir.dt.float32
    bf16 = mybir.dt.bfloat16
    fp8 = mybir.dt.float8e4
    AF = mybir.ActivationFunctionType
    pin = nc.declare_dram_parameter("p", [N_TILES, P, TILE_F], bf16, isOutput=False)
    gin = nc.declare_dram_parameter("g", [N_TILES, P, TILE_F], fp8, isOutput=False)
    out = nc.declare_dram_parameter("out", [N_TILES, P, TILE_F], bf16, isOutput=True)
    with TileContext(nc) as tc:
        with tc.tile_pool(name="consts", bufs=1) as cpool, \
             tc.tile_pool(name="sb", bufs=5) as pool:
            bias_ars = cpool.tile([P, 1], f32, tag="bias_ars")
            nc.gpsimd.memset(bias_ars[:], b_ars)
            for i in range(N_TILES):
                pt = pool.tile([P, TILE_F], bf16, tag="p")
                gt = pool.tile([P, TILE_F], fp8, tag="g")
                nc.sync.dma_start(out=pt[:], in_=pin[i])
                nc.sync.dma_start(out=gt[:], in_=gin[i])
                a = pool.tile([P, TILE_F], bf16, tag="a")
                b = pool.tile([P, TILE_F], bf16, tag="b")
                # Both pointwise table ops stay on ACT (one table set);
                # GpSimd compute would steal DVE's SBUF ports.
                nc.scalar.activation(a[:], gt[:], AF.Square)
                nc.scalar.activation(b[:], a[:], AF.Abs_reciprocal_sqrt,
                                     scale=ars_scale, bias=bias_ars[:])
                u = pool.tile([P, TILE_F], bf16, tag="u")
                nc.vector.tensor_mul(u[:], gt[:], b[:])
                ot = pool.tile([P, TILE_F], bf16, tag="o")
                nc.vector.tensor_sub(ot[:], pt[:], u[:])
                nc.gpsimd.dma_start(out=out[i], in_=ot[:])
    nc.finalize()
    return nc


def _build_general(k_sq, v_scale, m_scale):
    """out = p - (m_scale*m + g) / sqrt((k_sq*g)^2 + v_scale*v)."""
    nc = bacc.Bacc(None, target_bir_lowering=False)
    f32 = mybir.dt.float32
    AF = mybir.ActivationFunctionType
    ALU = mybir.AluOpType
    pin = nc.declare_dram_parameter("p", [N_TILES, P, TILE_F], f32, isOutput=False)
    gin = nc.declare_dram_parameter("g", [N_TILES, P, TILE_F], f32, isOutput=False)
    min_ = nc.declare_dram_parameter("m", [N_TILES, P, TILE_F], f32, isOutput=False)
    vin = nc.declare_dram_parameter("v", [N_TILES, P, TILE_F], f32, isOutput=False)
    out = nc.declare_dram_parameter("out", [N_TILES, P, TILE_F], f32, isOutput=True)
    with TileContext(nc) as tc:
        with tc.tile_pool(name="sb", bufs=3) as pool:
            for i in range(N_TILES):
                pt = pool.tile([P, TILE_F], f32, tag="p")
                gt = pool.tile([P, TILE_F], f32, tag="g")
                mt = pool.tile([P, TILE_F], f32, tag="m")
                vt = pool.tile([P, TILE_F], f32, tag="v")
                nc.sync.dma_start(out=pt[:], in_=pin[i])
                nc.sync.dma_start(out=gt[:], in_=gin[i])
                nc.sync.dma_start(out=mt[:], in_=min_[i])
                nc.sync.dma_start(out=vt[:], in_=vin[i])
                a = pool.tile([P, TILE_F], f32, tag="a")
                b = pool.tile([P, TILE_F], f32, tag="b")
                nc.scalar.activation(a[:], gt[:], AF.Square, scale=k_sq)
                # b = v*v_scale + a
                nc.vector.scalar_tensor_tensor(b[:], vt[:], v_scale, a[:],
                                               ALU.mult, ALU.add)
                nc.scalar.activation(a[:], b[:], AF.Abs_reciprocal_sqrt)
                # b = m*m_scale + g
                nc.vector.scalar_tensor_tensor(b[:], mt[:], m_scale, gt[:],
                                               ALU.mult, ALU.add)
                nc.vector.tensor_mul(a[:], b[:], a[:])
                ot = pool.tile([P, TILE_F], f32, tag="o")
                nc.vector.tensor_sub(ot[:], pt[:], a[:])
                nc.scalar.dma_start(out=out[i], in_=ot[:])
    nc.finalize()
    return nc


def kernel(alpha, beta1_raw, beta2_raw, log_eps,
           param_conv, grad_conv, m_conv, v_conv,
           param_mlp, grad_mlp, m_mlp, v_mlp,
           param_head, grad_head, m_head, v_head, t):
    global LAST_RESULT
    alpha = float(np.asarray(alpha))
    beta1 = (math.tanh(float(np.asarray(beta1_raw))) + 1.0) / 2.0
    beta2 = (math.tanh(float(np.asarray(beta2_raw))) + 1.0) / 2.0
    eps = 10.0 ** float(np.asarray(log_eps))
    t = int(np.asarray(t))
    bc1 = 1.0 - beta1 ** t
    bc2 = 1.0 - beta2 ** t

    params = {"conv": (param_conv, grad_conv, m_conv, v_conv),
              "mlp": (param_mlp, grad_mlp, m_mlp, v_mlp),
              "head": (param_head, grad_head, m_head, v_head)}

    def flat(idx):
        return np.concatenate(
            [np.asarray(params[k][idx], dtype=np.float32).ravel() for k in _ORDER])

    p_flat = flat(0)
    g_flat = flat(1)
    m_flat = flat(2)
    v_flat = flat(3)

    # A: numerator coefficient on g; B: g^2 coefficient inside sqrt
    A = alpha * (1.0 - beta1) / bc1
    B = (1.0 - beta2) / bc2

    v0 = float(v_flat[0])
    fast = (not np.any(m_flat)) and bool(np.all(v_flat == v0))

    def shard(x, dtype=None):
        if dtype is not None:
            x = x.astype(dtype)
        return [np.ascontiguousarray(
            x[i * PER_CORE:(i + 1) * PER_CORE].reshape(N_TILES, P, TILE_F))
            for i in range(N_CORES)]

    if fast:
        C = beta2 * v0 / bc2
        key = ("fast", A, B, C)
        if key not in _nc_cache:
            _nc_cache[key] = _build_fast(
                ars_scale=B / (A * A),
                b_ars=max(C * G8_SCALE * G8_SCALE / (A * A), 1e-30))
        nc = _nc_cache[key]
        bf = ml_dtypes.bfloat16
        ps = shard(p_flat, bf)
        gs = shard(g_flat * np.float32(G8_SCALE), ml_dtypes.float8_e4m3)
        in_maps = [{"p": ps[i], "g": gs[i]} for i in range(N_CORES)]
    else:
        D = beta2 / bc2
        key = ("gen", A, B, D, beta1)
        if key not in _nc_cache:
            _nc_cache[key] = _build_general(
                k_sq=math.sqrt(B) / A, v_scale=D / (A * A),
                m_scale=beta1 / (1.0 - beta1))
        nc = _nc_cache[key]
        ps, gs, ms, vs = shard(p_flat), shard(g_flat), shard(m_flat), shard(v_flat)
        in_maps = [{"p": ps[i], "g": gs[i], "m": ms[i], "v": vs[i]}
                   for i in range(N_CORES)]

    res = run_bass_kernel_spmd(nc, in_maps, core_ids=list(range(N_CORES)),
                               trace=TRACE)
    LAST_RESULT = res
    return np.concatenate(
        [res.results[i]["out"].astype(np.float32).reshape(-1)
         for i in range(N_CORES)])


# revision 21
# speedup vs baseline: 1.0341x; 1.0341x over previous
"""Distributed Adam optimizer step on 8 TRN2 NeuronCores.

Computes the Adam parameter patch for three tensors (conv/mlp/head),
returning the flat concatenation exactly like the reference.

Strategy (pure data-parallel, ZeRO-style): all tensors are flattened and
concatenated into one flat stream of 23,232,512 f32 elements, split evenly
across the 8 cores (2,904,064 each). Each core runs an identical elementwise
Bass/Tile kernel over its chunk; no collectives needed. Scalar hyperparams
are folded on the host into activation scale/bias immediates.

If the moment tensors are degenerate (m == 0 everywhere, v constant — the
case at t=1), an exact algebraic specialization skips loading m and v,
cutting HBM traffic from 5 streams to 3.
"""

import math

import ml_dtypes
import numpy as np

import concourse.bacc as bacc
import concourse.mybir as mybir
from concourse.tile import TileContext
from concourse.bass_utils import run_bass_kernel_spmd

N_CORES = 8
TOTAL = 512 * 512 * 3 * 3 + 4096 * 4096 + 1000 * 4096  # 23,232,512
PER_CORE = TOTAL // N_CORES  # 2,904,064
P = 128
TILE_F = 1418
N_TILES = PER_CORE // (P * TILE_F)  # 16
assert N_TILES * P * TILE_F == PER_CORE

_ORDER = ("conv", "mlp", "head")

TRACE = False
LAST_RESULT = None

_nc_cache = {}

# The act-table placement pass assigns each ACTIVATE the first table set
# containing its function; Square would first-fit to "exp_and_others" while
# Abs_reciprocal_sqrt lives in "abs_reciprocal_sqrt_and_small", which would
# reload tables twice per tile (~2.6us each). Both functions coexist in
# abs_reciprocal_sqrt_and_small; hide them from every other set (order and
# set count preserved, so act_func_set_ids stay valid) and the whole kernel
# needs exactly one table load.
_orig_get_activation_tables = bacc.get_activation_tables


def _patched_get_activation_tables(arch):
    tables = dict(_orig_get_activation_tables(arch))
    AF = mybir.ActivationFunctionType
    pinned = {AF.Square, AF.Abs_reciprocal_sqrt}
    out = {}
    for name, funcs in tables.items():
        if name == "abs_reciprocal_sqrt_and_small":
            out[name] = funcs
        else:
            out[name] = funcs - pinned
    return out


bacc.get_activation_tables = _patched_get_activation_tables


def _build_fast(k_sq, b_ars):
    """out = p - g / sqrt((k_sq*g)^2 + b_ars), all I/O in bf16.

    Exact Adam patch (modulo the +eps in the denominator, which perturbs
    the update term by <0.4% only where |g| is tiny) when m==0 and
    v==const; all scalars folded into k_sq/b_ars. bf16 streams halve HBM
    traffic (the binding resource) and unlock the DVE 2x perf mode; the
    quantization adds ~1e-3 norm relative error, well inside the 2e-2
    gate. The rsqrt is the Abs_reciprocal_sqrt ACT table function
    (1 elem/cycle) instead of DVE reciprocal (~6 cycles/elem)."""
    nc = bacc.Bacc(None, target_bir_lowering=False)
    f32 = mybir.dt.float32
    bf16 = mybir.dt.bfloat16
    AF = mybir.ActivationFunctionType
    pin = nc.declare_dram_parameter("p", [N_TILES, P, TILE_F], bf16, isOutput=False)
    gin = nc.declare_dram_parameter("g", [N_TILES, P, TILE_F], bf16, isOutput=False)
    out = nc.declare_dram_parameter("out", [N_TILES, P, TILE_F], bf16, isOutput=True)
    ALU = mybir.AluOpType
    with TileContext(nc) as tc:
        with tc.tile_pool(name="consts", bufs=1) as cpool, \
             tc.tile_pool(name="sb", bufs=8) as pool:
            bias_ars = cpool.tile([P, 1], f32, tag="bias_ars")
            nc.gpsimd.memset(bias_ars[:], b_ars)
            for i in range(N_TILES):
                gt = pool.tile([P, TILE_F], bf16, tag="g")
                pt = pool.tile([P, TILE_F], bf16, tag="p")
                # g first (the compute chain starts from it); p on a
                # separate HWDGE queue so the two load streams and the
                # store stream ride three different queues.
                nc.sync.dma_start(out=gt[:], in_=gin[i])
                nc.scalar.dma_start(out=pt[:], in_=pin[i])
                a = pool.tile([P, TILE_F], bf16, tag="a")
                b = pool.tile([P, TILE_F], bf16, tag="b")
                # Square on DVE (all-bf16 keeps the 2x perf mode); ACT only
                # runs the rsqrt table op, with k_sq^2 folded into its input
                # scale. GpSimd compute would steal DVE's SBUF ports.
                nc.vector.tensor_mul(a[:], gt[:], gt[:])
                nc.scalar.activation(b[:], a[:], AF.Abs_reciprocal_sqrt,
                                     scale=k_sq * k_sq, bias=bias_ars[:])
                u = pool.tile([P, TILE_F], bf16, tag="u")
                nc.vector.tensor_mul(u[:], gt[:], b[:])
                ot = pool.tile([P, TILE_F], bf16, tag="o")
                nc.vector.tensor_sub(ot[:], pt[:], u[:])
                nc.gpsimd.dma_start(out=out[i], in_=ot[:])
    nc.finalize()
    return nc


def _build_general(k_sq, v_scale, m_scale):
    """out = p - (m_scale*m + g) / sqrt((k_sq*g)^2 + v_scale*v)."""
    nc = bacc.Bacc(None, target_bir_lowering=False)
    f32 = mybir.dt.float32
    AF = mybir.ActivationFunctionType
    ALU = mybir.AluOpType
    pin = nc.declare_dram_parameter("p", [N_TILES, P, TILE_F], f32, isOutput=False)
    gin = nc.declare_dram_parameter("g", [N_TILES, P, TILE_F], f32, isOutput=False)
    min_ = nc.declare_dram_parameter("m", [N_TILES, P, TILE_F], f32, isOutput=False)
    vin = nc.declare_dram_parameter("v", [N_TILES, P, TILE_F], f32, isOutput=False)
    out = nc.declare_dram_parameter("out", [N_TILES, P, TILE_F], f32, isOutput=True)
    with TileContext(nc) as tc:
        with tc.tile_pool(name="sb", bufs=3) as pool:
            for i in range(N_TILES):
                pt = pool.tile([P, TILE_F], f32, tag="p")
                gt = pool.tile([P, TILE_F], f32, tag="g")
                mt = pool.tile([P, TILE_F], f32, tag="m")
                vt = pool.tile([P, TILE_F], f32, tag="v")
                nc.sync.dma_start(out=pt[:], in_=pin[i])
                nc.sync.dma_start(out=gt[:], in_=gin[i])
                nc.sync.dma_start(out=mt[:], in_=min_[i])
                nc.sync.dma_start(out=vt[:], in_=vin[i])
                a = pool.tile([P, TILE_F], f32, tag="a")
                b = pool.tile([P, TILE_F], f32, tag="b")
                nc.scalar.activation(a[:], gt[:], AF.Square, scale=k_sq)
                # b = v*v_scale + a
                nc.vector.scalar_tensor_tensor(b[:], vt[:], v_scale, a[:],
                                               ALU.mult, ALU.add)
                nc.scalar.activation(a[:], b[:], AF.Abs_reciprocal_sqrt)
                # b = m*m_scale + g
                nc.vector.scalar_tensor_tensor(b[:], mt[:], m_scale, gt[:],
                                               ALU.mult, ALU.add)
                nc.vector.tensor_mul(a[:], b[:], a[:])
                ot = pool.tile([P, TILE_F], f32, tag="o")
                nc.vector.tensor_sub(ot[:], pt[:], a[:])
                nc.scalar.dma_start(out=out[i], in_=ot[:])
    nc.finalize()
    return nc


def kernel(alpha, beta1_raw, beta2_raw, log_eps,
           param_conv, grad_conv, m_conv, v_conv,
           param_mlp, grad_mlp, m_mlp, v_mlp,
           param_head, grad_head, m_head, v_head, t):
    global LAST_RESULT
    alpha = float(np.asarray(alpha))
    beta1 = (math.tanh(float(np.asarray(beta1_raw))) + 1.0) / 2.0
    beta2 = (math.tanh(float(np.asarray(beta2_raw))) + 1.0) / 2.0
    eps = 10.0 ** float(np.asarray(log_eps))
    t = int(np.asarray(t))
    bc1 = 1.0 - beta1 ** t
    bc2 = 1.0 - beta2 ** t

    params = {"conv": (param_conv, grad_conv, m_conv, v_conv),
              "mlp": (param_mlp, grad_mlp, m_mlp, v_mlp),
              "head": (param_head, grad_head, m_head, v_head)}

    def flat(idx):
        return np.concatenate(
            [np.asarray(params[k][idx], dtype=np.float32).ravel() for k in _ORDER])

    p_flat = flat(0)
    g_flat = flat(1)
    m_flat = flat(2)
    v_flat = flat(3)

    # A: numerator coefficient on g; B: g^2 coefficient inside sqrt
    A = alpha * (1.0 - beta1) / bc1
    B = (1.0 - beta2) / bc2

    v0 = float(v_flat[0])
    fast = (not np.any(m_flat)) and bool(np.all(v_flat == v0))

    def shard(x, dtype=None):
        if dtype is not None:
            x = x.astype(dtype)
        return [np.ascontiguousarray(
            x[i * PER_CORE:(i + 1) * PER_CORE].reshape(N_TILES, P, TILE_F))
            for i in range(N_CORES)]

    if fast:
        C = beta2 * v0 / bc2
        key = ("fast", A, B, C)
        if key not in _nc_cache:
            _nc_cache[key] = _build_fast(
                k_sq=math.sqrt(B) / A, b_ars=max(C / (A * A), 1e-30))
        nc = _nc_cache[key]
        bf = ml_dtypes.bfloat16
        ps, gs = shard(p_flat, bf), shard(g_flat, bf)
        in_maps = [{"p": ps[i], "g": gs[i]} for i in range(N_CORES)]
    else:
        D = beta2 / bc2
        key = ("gen", A, B, D, beta1)
        if key not in _nc_cache:
            _nc_cache[key] = _build_general(
                k_sq=math.sqrt(B) / A, v_scale=D / (A * A),
                m_scale=beta1 / (1.0 - beta1))
        nc = _nc_cache[key]
        ps, gs, ms, vs = shard(p_flat), shard(g_flat), shard(m_flat), shard(v_flat)
        in_maps = [{"p": ps[i], "g": gs[i], "m": ms[i], "v": vs[i]}
                   for i in range(N_CORES)]

    res = run_bass_kernel_spmd(nc, in_maps, core_ids=list(range(N_CORES)),
                               trace=TRACE)
    LAST_RESULT = res
    return np.concatenate(
        [res.results[i]["out"].astype(np.float32).reshape(-1)
         for i in range(N_CORES)])


# revision 26
# speedup vs baseline: 1.1483x; 1.1104x over previous
"""Distributed Adam optimizer step on 8 TRN2 NeuronCores.

Computes the Adam parameter patch for three tensors (conv/mlp/head),
returning the flat concatenation exactly like the reference.

Strategy (pure data-parallel, ZeRO-style): all tensors are flattened and
concatenated into one flat stream of 23,232,512 f32 elements, split evenly
across the 8 cores (2,904,064 each). Each core runs an identical elementwise
Bass/Tile kernel over its chunk; no collectives needed. Scalar hyperparams
are folded on the host into activation scale/bias immediates.

If the moment tensors are degenerate (m == 0 everywhere, v constant — the
case at t=1), an exact algebraic specialization skips loading m and v,
cutting HBM traffic from 5 streams to 3.
"""

import math

import ml_dtypes
import numpy as np

import concourse.bacc as bacc
import concourse.mybir as mybir
from concourse.tile import TileContext
from concourse.bass_utils import run_bass_kernel_spmd

N_CORES = 8
TOTAL = 512 * 512 * 3 * 3 + 4096 * 4096 + 1000 * 4096  # 23,232,512
PER_CORE = TOTAL // N_CORES  # 2,904,064
P = 128
TILE_F = 1418
N_TILES = PER_CORE // (P * TILE_F)  # 16
assert N_TILES * P * TILE_F == PER_CORE

_ORDER = ("conv", "mlp", "head")

TRACE = False
USE_RAW = True
LAST_RESULT = None

_nc_cache = {}

# The act-table placement pass assigns each ACTIVATE the first table set
# containing its function; Square would first-fit to "exp_and_others" while
# Abs_reciprocal_sqrt lives in "abs_reciprocal_sqrt_and_small", which would
# reload tables twice per tile (~2.6us each). Both functions coexist in
# abs_reciprocal_sqrt_and_small; hide them from every other set (order and
# set count preserved, so act_func_set_ids stay valid) and the whole kernel
# needs exactly one table load.
_orig_get_activation_tables = bacc.get_activation_tables


def _patched_get_activation_tables(arch):
    tables = dict(_orig_get_activation_tables(arch))
    AF = mybir.ActivationFunctionType
    pinned = {AF.Square, AF.Abs_reciprocal_sqrt}
    out = {}
    for name, funcs in tables.items():
        if name == "abs_reciprocal_sqrt_and_small":
            out[name] = funcs
        else:
            out[name] = funcs - pinned
    return out


bacc.get_activation_tables = _patched_get_activation_tables


def _build_fast(k_sq, b_ars):
    """out = p - g / sqrt((k_sq*g)^2 + b_ars), all I/O in bf16.

    Exact Adam patch (modulo the +eps in the denominator, which perturbs
    the update term by <0.4% only where |g| is tiny) when m==0 and
    v==const; all scalars folded into k_sq/b_ars. bf16 streams halve HBM
    traffic (the binding resource) and unlock the DVE 2x perf mode; the
    quantization adds ~1e-3 norm relative error, well inside the 2e-2
    gate. The rsqrt is the Abs_reciprocal_sqrt ACT table function
    (1 elem/cycle) instead of DVE reciprocal (~6 cycles/elem)."""
    nc = bacc.Bacc(None, target_bir_lowering=False)
    f32 = mybir.dt.float32
    bf16 = mybir.dt.bfloat16
    AF = mybir.ActivationFunctionType
    pin = nc.declare_dram_parameter("p", [N_TILES, P, TILE_F], bf16, isOutput=False)
    gin = nc.declare_dram_parameter("g", [N_TILES, P, TILE_F], bf16, isOutput=False)
    out = nc.declare_dram_parameter("out", [N_TILES, P, TILE_F], bf16, isOutput=True)
    ALU = mybir.AluOpType
    with TileContext(nc) as tc:
        with tc.tile_pool(name="consts", bufs=1) as cpool, \
             tc.tile_pool(name="sb", bufs=8) as pool:
            bias_ars = cpool.tile([P, 1], f32, tag="bias_ars")
            nc.gpsimd.memset(bias_ars[:], b_ars)
            for i in range(N_TILES):
                gt = pool.tile([P, TILE_F], bf16, tag="g")
                pt = pool.tile([P, TILE_F], bf16, tag="p")
                # g first (the compute chain starts from it); p on a
                # separate HWDGE queue so the two load streams and the
                # store stream ride three different queues.
                nc.sync.dma_start(out=gt[:], in_=gin[i])
                nc.scalar.dma_start(out=pt[:], in_=pin[i])
                a = pool.tile([P, TILE_F], bf16, tag="a")
                b = pool.tile([P, TILE_F], bf16, tag="b")
                # Square on DVE (all-bf16 keeps the 2x perf mode); ACT only
                # runs the rsqrt table op, with k_sq^2 folded into its input
                # scale. GpSimd compute would steal DVE's SBUF ports.
                nc.vector.tensor_mul(a[:], gt[:], gt[:])
                nc.scalar.activation(b[:], a[:], AF.Abs_reciprocal_sqrt,
                                     scale=k_sq * k_sq, bias=bias_ars[:])
                u = pool.tile([P, TILE_F], bf16, tag="u")
                nc.vector.tensor_mul(u[:], gt[:], b[:])
                ot = pool.tile([P, TILE_F], bf16, tag="o")
                nc.vector.tensor_sub(ot[:], pt[:], u[:])
                nc.gpsimd.dma_start(out=out[i], in_=ot[:])
    nc.finalize()
    return nc


def _build_fast_raw(ars_scale, b_ars):
    """Raw-bacc (no Tile) version of the fast path: hand-placed semaphores,
    cyclic SBUF buffers, software-pipelined engine streams. Avoids Tile's
    ~9us kernel-tail drain/barrier butterfly and scheduling slack.

    Engine plan per tile i (all tiles bf16):
      sync:   g-load(i), p-load(i)            (one in-order HWDGE queue)
      DVE:    sq(i)=g*g, mul(i)=g*r, sub(i)=p-u   (sq runs one tile ahead)
      ACT:    r(i) = 1/sqrt(ars_scale*sq + b_ars)  (Abs_reciprocal_sqrt)
      gpsimd: store(i)
    """
    from contextlib import ExitStack

    nc = bacc.Bacc(None, target_bir_lowering=False)
    f32 = mybir.dt.float32
    bf16 = mybir.dt.bfloat16
    AF = mybir.ActivationFunctionType
    N = N_TILES
    pin = nc.declare_dram_parameter("p", [N, P, TILE_F], bf16, isOutput=False)
    gin = nc.declare_dram_parameter("g", [N, P, TILE_F], bf16, isOutput=False)
    out = nc.declare_dram_parameter("out", [N, P, TILE_F], bf16, isOutput=True)

    K = 6   # load/store ring depth
    KI = 4  # intermediate ring depth

    # DVE stream: sq one tile ahead of mul/sub so the ACT round-trip for
    # tile i overlaps with squaring tile i+1.
    dve_ops = [("sq", 0)]
    for i in range(N):
        if i + 1 < N:
            dve_ops.append(("sq", i + 1))
        dve_ops.append(("mul", i))
        dve_ops.append(("sub", i))
    dve_pos = {op: k for k, op in enumerate(dve_ops)}

    with ExitStack() as st:
        gbuf = st.enter_context(nc.sbuf_tensor("gbuf", [P, K * TILE_F], bf16))
        pbuf = st.enter_context(nc.sbuf_tensor("pbuf", [P, K * TILE_F], bf16))
        abuf = st.enter_context(nc.sbuf_tensor("abuf", [P, KI * TILE_F], bf16))
        bbuf = st.enter_context(nc.sbuf_tensor("bbuf", [P, KI * TILE_F], bf16))
        ubuf = st.enter_context(nc.sbuf_tensor("ubuf", [P, KI * TILE_F], bf16))
        obuf = st.enter_context(nc.sbuf_tensor("obuf", [P, K * TILE_F], bf16))
        bias_t = st.enter_context(nc.sbuf_tensor("ars_bias", [P, 1], f32))
        sem_g = st.enter_context(nc.semaphore("sem_g"))
        sem_p = st.enter_context(nc.semaphore("sem_p"))
        sem_st = st.enter_context(nc.semaphore("sem_st"))
        sem_act = st.enter_context(nc.semaphore("sem_act"))
        sem_dve = st.enter_context(nc.semaphore("sem_dve"))
        sem_bias = st.enter_context(nc.semaphore("sem_bias"))
        block = st.enter_context(nc.Block())

        def sl(buf, i, depth):
            j = i % depth
            return buf.ap()[:, j * TILE_F:(j + 1) * TILE_F]

        @block.sync
        def _(sync):
            for i in range(N):
                if i >= K:
                    # g ring slot free once mul(i-K) has read it
                    sync.wait_ge(sem_dve, dve_pos[("mul", i - K)] + 1)
                sync.dma_start(out=sl(gbuf, i, K), in_=gin[i]).then_inc(sem_g, 16)
                if i >= K:
                    sync.wait_ge(sem_dve, dve_pos[("sub", i - K)] + 1)
                sync.dma_start(out=sl(pbuf, i, K), in_=pin[i]).then_inc(sem_p, 16)

        @block.vector
        def _(vector):
            for kind, i in dve_ops:
                if kind == "sq":
                    vector.wait_ge(sem_g, 16 * (i + 1))
                    if i >= KI:
                        # a slot free once ars(i-KI) has read it
                        vector.wait_ge(sem_act, i - KI + 1)
                    vector.tensor_mul(sl(abuf, i, KI), sl(gbuf, i, K),
                                      sl(gbuf, i, K)).then_inc(sem_dve, 1)
                elif kind == "mul":
                    vector.wait_ge(sem_act, i + 1)
                    vector.tensor_mul(sl(ubuf, i, KI), sl(gbuf, i, K),
                                      sl(bbuf, i, KI)).then_inc(sem_dve, 1)
                else:  # sub
                    vector.wait_ge(sem_p, 16 * (i + 1))
                    if i >= K:
                        vector.wait_ge(sem_st, 16 * (i - K + 1))
                    vector.tensor_sub(sl(obuf, i, K), sl(pbuf, i, K),
                                      sl(ubuf, i, KI)).then_inc(sem_dve, 1)

        @block.scalar
        def _(scalar):
            for i in range(N):
                scalar.wait_ge(sem_dve, dve_pos[("sq", i)] + 1)
                if i == 0:
                    scalar.wait_ge(sem_bias, 1)
                # b slot free once mul(i-KI) has read it — implied by the
                # sem_dve wait above (mul(i-KI) precedes sq(i) in dve_ops)
                scalar.activation(sl(bbuf, i, KI), sl(abuf, i, KI),
                                  AF.Abs_reciprocal_sqrt, scale=ars_scale,
                                  bias=bias_t.ap()).then_inc(sem_act, 1)

        @block.gpsimd
        def _(gpsimd):
            gpsimd.memset(bias_t.ap(), b_ars).then_inc(sem_bias, 1)
            for i in range(N):
                gpsimd.wait_ge(sem_dve, dve_pos[("sub", i)] + 1)
                gpsimd.dma_start(out=out[i], in_=sl(obuf, i, K)).then_inc(
                    sem_st, 16)
            gpsimd.wait_ge(sem_st, 16 * N)

    nc.finalize()
    return nc


def _build_general(k_sq, v_scale, m_scale):
    """out = p - (m_scale*m + g) / sqrt((k_sq*g)^2 + v_scale*v)."""
    nc = bacc.Bacc(None, target_bir_lowering=False)
    f32 = mybir.dt.float32
    AF = mybir.ActivationFunctionType
    ALU = mybir.AluOpType
    pin = nc.declare_dram_parameter("p", [N_TILES, P, TILE_F], f32, isOutput=False)
    gin = nc.declare_dram_parameter("g", [N_TILES, P, TILE_F], f32, isOutput=False)
    min_ = nc.declare_dram_parameter("m", [N_TILES, P, TILE_F], f32, isOutput=False)
    vin = nc.declare_dram_parameter("v", [N_TILES, P, TILE_F], f32, isOutput=False)
    out = nc.declare_dram_parameter("out", [N_TILES, P, TILE_F], f32, isOutput=True)
    with TileContext(nc) as tc:
        with tc.tile_pool(name="sb", bufs=3) as pool:
            for i in range(N_TILES):
                pt = pool.tile([P, TILE_F], f32, tag="p")
                gt = pool.tile([P, TILE_F], f32, tag="g")
                mt = pool.tile([P, TILE_F], f32, tag="m")
                vt = pool.tile([P, TILE_F], f32, tag="v")
                nc.sync.dma_start(out=pt[:], in_=pin[i])
                nc.sync.dma_start(out=gt[:], in_=gin[i])
                nc.sync.dma_start(out=mt[:], in_=min_[i])
                nc.sync.dma_start(out=vt[:], in_=vin[i])
                a = pool.tile([P, TILE_F], f32, tag="a")
                b = pool.tile([P, TILE_F], f32, tag="b")
                nc.scalar.activation(a[:], gt[:], AF.Square, scale=k_sq)
                # b = v*v_scale + a
                nc.vector.scalar_tensor_tensor(b[:], vt[:], v_scale, a[:],
                                               ALU.mult, ALU.add)
                nc.scalar.activation(a[:], b[:], AF.Abs_reciprocal_sqrt)
                # b = m*m_scale + g
                nc.vector.scalar_tensor_tensor(b[:], mt[:], m_scale, gt[:],
                                               ALU.mult, ALU.add)
                nc.vector.tensor_mul(a[:], b[:], a[:])
                ot = pool.tile([P, TILE_F], f32, tag="o")
                nc.vector.tensor_sub(ot[:], pt[:], a[:])
                nc.scalar.dma_start(out=out[i], in_=ot[:])
    nc.finalize()
    return nc


def kernel(alpha, beta1_raw, beta2_raw, log_eps,
           param_conv, grad_conv, m_conv, v_conv,
           param_mlp, grad_mlp, m_mlp, v_mlp,
           param_head, grad_head, m_head, v_head, t):
    global LAST_RESULT
    alpha = float(np.asarray(alpha))
    beta1 = (math.tanh(float(np.asarray(beta1_raw))) + 1.0) / 2.0
    beta2 = (math.tanh(float(np.asarray(beta2_raw))) + 1.0) / 2.0
    eps = 10.0 ** float(np.asarray(log_eps))
    t = int(np.asarray(t))
    bc1 = 1.0 - beta1 ** t
    bc2 = 1.0 - beta2 ** t

    params = {"conv": (param_conv, grad_conv, m_conv, v_conv),
              "mlp": (param_mlp, grad_mlp, m_mlp, v_mlp),
              "head": (param_head, grad_head, m_head, v_head)}

    def flat(idx):
        return np.concatenate(
            [np.asarray(params[k][idx], dtype=np.float32).ravel() for k in _ORDER])

    p_flat = flat(0)
    g_flat = flat(1)
    m_flat = flat(2)
    v_flat = flat(3)

    # A: numerator coefficient on g; B: g^2 coefficient inside sqrt
    A = alpha * (1.0 - beta1) / bc1
    B = (1.0 - beta2) / bc2

    v0 = float(v_flat[0])
    fast = (not np.any(m_flat)) and bool(np.all(v_flat == v0))

    def shard(x, dtype=None):
        if dtype is not None:
            x = x.astype(dtype)
        return [np.ascontiguousarray(
            x[i * PER_CORE:(i + 1) * PER_CORE].reshape(N_TILES, P, TILE_F))
            for i in range(N_CORES)]

    if fast:
        C = beta2 * v0 / bc2
        key = ("fast", A, B, C, USE_RAW)
        if key not in _nc_cache:
            if USE_RAW:
                _nc_cache[key] = _build_fast_raw(
                    ars_scale=B / (A * A),
                    b_ars=max(C / (A * A), 1e-30))
            else:
                _nc_cache[key] = _build_fast(
                    k_sq=math.sqrt(B) / A, b_ars=max(C / (A * A), 1e-30))
        nc = _nc_cache[key]
        bf = ml_dtypes.bfloat16
        ps, gs = shard(p_flat, bf), shard(g_flat, bf)
        in_maps = [{"p": ps[i], "g": gs[i]} for i in range(N_CORES)]
    else:
        D = beta2 / bc2
        key = ("gen", A, B, D, beta1)
        if key not in _nc_cache:
            _nc_cache[key] = _build_general(
                k_sq=math.sqrt(B) / A, v_scale=D / (A * A),
                m_scale=beta1 / (1.0 - beta1))
        nc = _nc_cache[key]
        ps, gs, ms, vs = shard(p_flat), shard(g_flat), shard(m_flat), shard(v_flat)
        in_maps = [{"p": ps[i], "g": gs[i], "m": ms[i], "v": vs[i]}
                   for i in range(N_CORES)]

    res = run_bass_kernel_spmd(nc, in_maps, core_ids=list(range(N_CORES)),
                               trace=TRACE)
    LAST_RESULT = res
    return np.concatenate(
        [res.results[i]["out"].astype(np.float32).reshape(-1)
         for i in range(N_CORES)])
